# revision 3
# baseline (speedup 1.0000x reference)
"""BiLSTM-CRF Trainium2 kernel (8-core SPMD, direction-split data parallel).

Sharding: 8 cores; core c < 4 runs the FORWARD LSTM for sentences
[16c, 16c+16); core c+4 runs the BACKWARD LSTM for the same sentences
(fed a time-reversed sentence).  Each core: embedding gather (indirect
DMA), input projection (PE), sequential LSTM scan (PE + ACT + DVE),
per-step tag-projection partials, then a 2-core AllGather exchanges the
two direction partials, each core forms full CRF emission scores and
runs the Viterbi forward recursion.  Host does the (trivial) backtrace
from the device dp trajectory.

Layouts (per core, B=16 local sentences, H=256/dir, G=1024 gates):
  - state kept transposed: hT[128*2, B]; gates PSUM [128, (mi, b)]
  - gate blocks permuted to (g, i, f, o) so tanh/sigmoid are 2 ops
  - weights pre-transposed on host into matmul lhsT layout
  - Viterbi dp chain on partitions 0:16 (b), free = (kn, kp)
"""

import os
from dataclasses import dataclass, field

import numpy as np

NEG = -10000.0
START, STOP = 10, 11
K = 12
E = 256
H = 256
G = 1024
VOCAB = 100000
T_FULL = 512
B_GLOB = 64
B = 16  # per-core sentences

# gate permutation: original rows [i(0:256), f(256:512), g(512:768), o(768:1024)]
# new block order (g, i, f, o)
GATE_PERM = np.concatenate(
    [np.arange(512, 768), np.arange(0, 256), np.arange(256, 512), np.arange(768, 1024)]
)


@dataclass
class Cfg:
    T: int = 512
    SC: int = 16            # scan chunk (steps)
    PC: int = 1024          # projection chunk (tokens)
    vocab: int = VOCAB
    n_cores: int = 8
    groups: tuple = ((0, 4), (1, 5), (2, 6), (3, 7))
    # (direction, batch-slice-start) per core
    cores: tuple = ((0, 0), (0, 16), (0, 32), (0, 48), (1, 0), (1, 16), (1, 32), (1, 48))
    fake_cc: bool = False   # replace AllGather with local copies (timing sims only)

    @property
    def RING(self):
        return 2 * self.SC

    @property
    def n_groups(self):
        return self.T * B // 128


# --------------------------------------------------------------------------
# device kernel body
# --------------------------------------------------------------------------

def build_body(nc, tc, ins, outs, cfg: Cfg):
    import concourse.bass as bass
    import concourse.mybir as mybir
    from concourse.masks import make_identity

    f32 = mybir.dt.float32
    Alu = mybir.AluOpType
    Act = mybir.ActivationFunctionType
    T, SC, PC, RING = cfg.T, cfg.SC, cfg.PC, cfg.RING
    NPC = min(512, PC)  # matmul N per projection instruction

    # internal DRAM
    xbuf = nc.dram_tensor("xbuf", [8, 128, T * B], f32)
    cc_in = nc.dram_tensor("cc_in", [B, T * K], f32)
    cc_out = nc.dram_tensor("cc_out", [2 * B, T * K], f32)

    with (
        tc.tile_pool(name="const", bufs=1) as constp,
        tc.tile_pool(name="persist", bufs=1) as pers,
        tc.tile_pool(name="emb", bufs=8) as embp,
        tc.tile_pool(name="embT", bufs=2) as embTp,
        tc.tile_pool(name="xsb", bufs=4) as xsbp,
        tc.tile_pool(name="xc", bufs=2) as xcp,
        tc.tile_pool(name="gsum", bufs=2) as gsump,
        tc.tile_pool(name="actt", bufs=2) as actp,
        tc.tile_pool(name="cc", bufs=2) as ccp,
        tc.tile_pool(name="crf", bufs=2) as crfp,
        tc.tile_pool(name="ps_g", bufs=2, space="PSUM") as ps_g,
        tc.tile_pool(name="ps_f", bufs=2, space="PSUM") as ps_f,
        tc.tile_pool(name="ps_x", bufs=2, space="PSUM") as ps_x,
        tc.tile_pool(name="ps_t", bufs=2, space="PSUM") as ps_t,
    ):
        # ---- load constants / weights into SBUF
        ident = constp.tile([128, 128], f32)
        make_identity(nc, ident[:])
        w_ihT = constp.tile([128, 2 * G], f32)
        nc.sync.dma_start(out=w_ihT[:], in_=ins["w_ihT"])
        w_hhT = constp.tile([128, 2 * G], f32)
        nc.sync.dma_start(out=w_hhT[:], in_=ins["w_hhT"])
        bias_t = constp.tile([128, 8], f32)
        nc.sync.dma_start(out=bias_t[:], in_=ins["bias_t"])
        w_outT = constp.tile([128, 2 * K], f32)
        nc.sync.dma_start(out=w_outT[:], in_=ins["w_outT"])
        bout = constp.tile([B, K], f32)
        nc.sync.dma_start(out=bout[:], in_=ins["bout"])
        transB = constp.tile([B, K * K], f32)
        nc.sync.dma_start(out=transB[:], in_=ins["transB"])
        dp0 = constp.tile([B, K], f32)
        nc.sync.dma_start(out=dp0[:], in_=ins["dp0"])
        idx_sb = constp.tile([128, cfg.n_groups], mybir.dt.int32)
        nc.sync.dma_start(out=idx_sb[:], in_=ins["idx"])

        # ---- persistent state
        hring = pers.tile([128, 2 * RING * B], f32)
        hring4 = hring[:].rearrange("p (k r b) -> p k r b", k=2, r=RING)
        h0 = pers.tile([128, 2 * B], f32)
        nc.gpsimd.memset(h0[:], 0)
        c0 = pers.tile([128, 2 * B], f32)
        nc.gpsimd.memset(c0[:], 0)
        partial = pers.tile([B, T * K], f32)
        fga = pers.tile([B, T * K], f32)
        fgb = pers.tile([B, T * K], f32)
        ftb = pers.tile([B, T * K], f32)
        dp_all = pers.tile([B, T * K], f32)

        # ================= phase 1: gather + input projection =================
        n_pc = (T * B) // PC
        for pc in range(n_pc):
            embT = embTp.tile([128, 2 * PC], f32, tag="embT")
            for j in range(PC // 128):
                g = pc * (PC // 128) + j
                emb_t = embp.tile([128, E], f32, tag="emb")
                nc.gpsimd.indirect_dma_start(
                    out=emb_t[:],
                    out_offset=None,
                    in_=ins["table"],
                    in_offset=bass.IndirectOffsetOnAxis(ap=idx_sb[:, g:g + 1], axis=0),
                )
                for eh in range(2):
                    tp = ps_t.tile([128, 128], f32, space="PSUM", tag="tp")
                    nc.tensor.transpose(
                        tp[:], emb_t[:, eh * 128:(eh + 1) * 128], ident[:]
                    )
                    nc.vector.tensor_copy(
                        embT[:, eh * PC + j * 128: eh * PC + (j + 1) * 128], tp[:]
                    )
            for mi in range(8):
                for nh in range(PC // NPC):
                    xp = ps_x.tile([128, NPC], f32, space="PSUM", tag="xp")
                    for ki in range(2):
                        nc.tensor.matmul(
                            xp[:],
                            lhsT=w_ihT[:, ki * G + mi * 128: ki * G + (mi + 1) * 128],
                            rhs=embT[:, ki * PC + nh * NPC: ki * PC + (nh + 1) * NPC],
                            start=(ki == 0),
                            stop=(ki == 1),
                        )
                    xsb = xsbp.tile([128, NPC], f32, tag="xsb")
                    nc.scalar.activation(
                        xsb[:], xp[:], Act.Identity, bias=bias_t[:, mi:mi + 1]
                    )
                    nc.sync.dma_start(
                        out=xbuf[mi, :, pc * PC + nh * NPC: pc * PC + (nh + 1) * NPC],
                        in_=xsb[:],
                    )

        # ================= phase 2: LSTM scan =================
        c_prev = c0
        f_ps = None
        for s in range(T):
            cs, sl = divmod(s, SC)
            if sl == 0:
                xc = xcp.tile([128, 8 * SC * B], f32, tag="xc")
                nc.sync.dma_start(
                    out=xc[:],
                    in_=xbuf[:, :, cs * SC * B:(cs + 1) * SC * B].rearrange(
                        "m p c -> p m c"
                    ),
                )
                xc4 = xc[:].rearrange("p (m s b) -> p m s b", m=8, s=SC)
                f_ps = ps_f.tile([B, SC * K], f32, space="PSUM", tag="fp")
            if s == 0:
                prev = h0[:].rearrange("p (k b) -> p k b", k=2)
            else:
                prev = hring4[:, :, (s - 1) % RING, :]
            g_ps = ps_g.tile([128, 8 * B], f32, space="PSUM", tag="gp")
            for mi in range(8):
                for ki in range(2):
                    nc.tensor.matmul(
                        g_ps[:, mi * B:(mi + 1) * B],
                        lhsT=w_hhT[:, ki * G + mi * 128: ki * G + (mi + 1) * 128],
                        rhs=prev[:, ki, :],
                        start=(ki == 0),
                        stop=(ki == 1),
                    )
            gsum = gsump.tile([128, 8 * B], f32, tag="gs")
            nc.vector.tensor_tensor(
                out=gsum[:].rearrange("p (m b) -> p m b", m=8),
                in0=g_ps[:].rearrange("p (m b) -> p m b", m=8),
                in1=xc4[:, :, sl, :],
                op=Alu.add,
            )
            th_g = actp.tile([128, 2 * B], f32, tag="thg")
            nc.scalar.activation(th_g[:], gsum[:, 0: 2 * B], Act.Tanh)
            sig = actp.tile([128, 6 * B], f32, tag="sig")
            nc.scalar.activation(sig[:], gsum[:, 2 * B: 8 * B], Act.Sigmoid)
            t1 = actp.tile([128, 2 * B], f32, tag="t1")
            nc.vector.tensor_tensor(
                out=t1[:], in0=sig[:, 0: 2 * B], in1=th_g[:], op=Alu.mult
            )
            t2 = actp.tile([128, 2 * B], f32, tag="t2")
            nc.vector.tensor_tensor(
                out=t2[:], in0=c_prev[:], in1=sig[:, 2 * B: 4 * B], op=Alu.mult
            )
            c_new = ccp.tile([128, 2 * B], f32, tag="c")
            nc.vector.tensor_tensor(out=c_new[:], in0=t1[:], in1=t2[:], op=Alu.add)
            th_c = actp.tile([128, 2 * B], f32, tag="thc")
            nc.scalar.activation(th_c[:], c_new[:], Act.Tanh)
            hslot = hring4[:, :, s % RING, :]
            nc.vector.tensor_tensor(
                out=hslot,
                in0=sig[:, 4 * B: 6 * B].rearrange("p (k b) -> p k b", k=2),
                in1=th_c[:].rearrange("p (k b) -> p k b", k=2),
                op=Alu.mult,
            )
            for ki in range(2):
                nc.tensor.matmul(
                    f_ps[:, sl * K:(sl + 1) * K],
                    lhsT=hring4[:, ki, s % RING, :],
                    rhs=w_outT[:, ki * K:(ki + 1) * K],
                    start=(ki == 0),
                    stop=(ki == 1),
                )
            c_prev = c_new
            if sl == SC - 1:
                nc.vector.tensor_copy(
                    out=partial[:, cs * SC * K:(cs + 1) * SC * K], in_=f_ps[:]
                )

        # ================= phase 3: exchange + emission scores =================
        nc.sync.dma_start(out=cc_in[:, :], in_=partial[:])
        if cfg.fake_cc:
            nc.sync.dma_start(out=cc_out[0:B, :], in_=cc_in[:, :])
            nc.sync.dma_start(out=cc_out[B: 2 * B, :], in_=cc_in[:, :])
        else:
            nc.gpsimd.collective_compute(
                "AllGather",
                mybir.AluOpType.bypass,
                replica_groups=[list(g) for g in cfg.groups],
                ins=[cc_in[:, :]],
                outs=[cc_out[:, :]],
            )
        nc.sync.dma_start(out=fga[:], in_=cc_out[0:B, :])
        nc.sync.dma_start(out=fgb[:], in_=cc_out[B: 2 * B, :])
        fgb_rev = fgb[:].rearrange("p (t k) -> p t k", k=K)[:, ::-1, :]
        nc.vector.tensor_tensor(
            out=ftb[:].rearrange("p (t k) -> p t k", k=K),
            in0=fga[:].rearrange("p (t k) -> p t k", k=K),
            in1=fgb_rev,
            op=Alu.add,
        )
        nc.vector.tensor_tensor(
            out=ftb[:].rearrange("p (t k) -> p t k", k=K),
            in0=ftb[:].rearrange("p (t k) -> p t k", k=K),
            in1=bout[:, None, :].to_broadcast([B, T, K]),
            op=Alu.add,
        )

        # ================= phase 4: Viterbi forward =================
        transB3 = transB[:].rearrange("p (a b) -> p a b", a=K)
        dp_prev = dp0[:]
        for t in range(T):
            sc_t = crfp.tile([B, K * K], f32, tag="sc")
            nc.vector.tensor_tensor(
                out=sc_t[:].rearrange("p (a b) -> p a b", a=K),
                in0=dp_prev[:, None, :].to_broadcast([B, K, K]),
                in1=transB3,
                op=Alu.add,
            )
            m_t = crfp.tile([B, K], f32, tag="m")
            nc.vector.tensor_reduce(
                out=m_t[:],
                in_=sc_t[:].rearrange("p (a b) -> p a b", a=K),
                axis=mybir.AxisListType.X,
                op=Alu.max,
            )
            nc.vector.tensor_tensor(
                out=dp_all[:, t * K:(t + 1) * K],
                in0=m_t[:],
                in1=ftb[:, t * K:(t + 1) * K],
                op=Alu.add,
            )
            dp_prev = dp_all[:, t * K:(t + 1) * K]

        # ---- outputs
        nc.sync.dma_start(out=outs["dp_all"], in_=dp_all[:])
        nc.sync.dma_start(out=outs["ftb"], in_=ftb[:])


# --------------------------------------------------------------------------
# host-side input prep / output postprocess
# --------------------------------------------------------------------------

def prep_core_inputs(cfg: Cfg, sentence, embed_table, Wf_ih, Wf_hh, bf_ih, bf_hh,
                     Wb_ih, Wb_hh, bb_ih, bb_hh, W_out, b_out, transitions):
    """Returns list of per-core input dicts (numpy)."""
    T = cfg.T
    table = np.ascontiguousarray(embed_table, dtype=np.float32)
    trans = np.asarray(transitions, dtype=np.float32)
    in_maps = []
    for (dirn, bs) in cfg.cores:
        sent = np.asarray(sentence[bs:bs + B, :T], dtype=np.int64)
        if dirn == 1:
            sent = sent[:, ::-1]
        flat = sent.T.reshape(-1)  # (t, b) order
        idx_np = np.ascontiguousarray(
            flat.reshape(cfg.n_groups, 128).T, dtype=np.int32
        )
        W_ih = np.asarray(Wf_ih if dirn == 0 else Wb_ih, dtype=np.float32)
        W_hh = np.asarray(Wf_hh if dirn == 0 else Wb_hh, dtype=np.float32)
        b_ih = np.asarray(bf_ih if dirn == 0 else bb_ih, dtype=np.float32)
        b_hh = np.asarray(bf_hh if dirn == 0 else bb_hh, dtype=np.float32)
        W_ihp = W_ih[GATE_PERM]
        W_hhp = W_hh[GATE_PERM]
        biasp = (b_ih + b_hh)[GATE_PERM]

        w = W_ihp.T.reshape(2, 128, G)
        w_ihT = np.ascontiguousarray(np.concatenate([w[0], w[1]], axis=1))
        w = W_hhp.T.reshape(2, 128, G)
        w_hhT = np.ascontiguousarray(np.concatenate([w[0], w[1]], axis=1))
        bias_t = np.ascontiguousarray(biasp.reshape(8, 128).T)
        off = 0 if dirn == 0 else H
        wo = np.asarray(W_out, dtype=np.float32)[:, off:off + H].T.reshape(2, 128, K)
        w_outT = np.ascontiguousarray(np.concatenate([wo[0], wo[1]], axis=1))
        bout = np.tile(np.asarray(b_out, dtype=np.float32)[None, :], (B, 1))
        transB = np.tile(trans.reshape(1, K * K), (B, 1))
        dp0 = np.full((B, K), NEG, np.float32)
        dp0[:, START] = 0.0
        in_maps.append({
            "table": table,
            "idx": idx_np,
            "w_ihT": w_ihT,
            "w_hhT": w_hhT,
            "bias_t": bias_t,
            "w_outT": w_outT,
            "bout": np.ascontiguousarray(bout),
            "transB": np.ascontiguousarray(transB),
            "dp0": dp0,
        })
    return in_maps


def postprocess(cfg: Cfg, results, transitions):
    """results: list of per-core output dicts with dp_all/ftb [B, T*K]."""
    T = cfg.T
    trans = np.asarray(transitions, dtype=np.float32)
    n_slices = B_GLOB // B
    path_scores = np.zeros(B_GLOB, np.float32)
    path = np.zeros((B_GLOB, T), np.int32)
    dp0 = np.full((B, K), NEG, np.float32)
    dp0[:, START] = 0.0
    for g in range(n_slices):
        dp_all = results[g]["dp_all"].reshape(B, T, K)
        bs = cfg.cores[g][1]
        dp_final = dp_all[:, T - 1, :]
        terminal = (dp_final + trans[STOP][None, :]).astype(np.float32)
        best = np.argmax(terminal, axis=1)
        path_scores[bs:bs + B] = np.max(terminal, axis=1)
        tag = best
        path[bs:bs + B, T - 1] = tag
        ar = np.arange(B)
        for t in range(T - 1, 0, -1):
            dpm1 = dp_all[:, t - 1, :]
            scores = (dpm1[:, None, :] + trans[None, :, :]).astype(np.float32)
            tag = np.argmax(scores[ar, tag, :], axis=-1).astype(np.int32)
            path[bs:bs + B, t - 1] = tag
    return path_scores, path


# --------------------------------------------------------------------------
# numpy fallback (general sentence_len) and per-core reference for sim tests
# --------------------------------------------------------------------------

def _np_sigmoid(x):
    return (1.0 / (1.0 + np.exp(-x.astype(np.float32)))).astype(np.float32)


def _np_lstm(x_proj, W_hh):
    # x_proj [T, B, 4H] includes both biases; returns hs [T, B, H]
    T_, Bn, G_ = x_proj.shape
    Hh = G_ // 4
    h = np.zeros((Bn, Hh), np.float32)
    c = np.zeros((Bn, Hh), np.float32)
    hs = np.zeros((T_, Bn, Hh), np.float32)
    WT = W_hh.T.astype(np.float32)
    for t in range(T_):
        gv = (x_proj[t] + h @ WT).astype(np.float32)
        i = _np_sigmoid(gv[:, 0:Hh])
        f = _np_sigmoid(gv[:, Hh:2 * Hh])
        gg = np.tanh(gv[:, 2 * Hh:3 * Hh]).astype(np.float32)
        o = _np_sigmoid(gv[:, 3 * Hh:4 * Hh])
        c = (f * c + i * gg).astype(np.float32)
        h = (o * np.tanh(c)).astype(np.float32)
        hs[t] = h
    return hs


def numpy_reference(sentence, sentence_len, embed_table,
                    Wf_ih, Wf_hh, bf_ih, bf_hh,
                    Wb_ih, Wb_hh, bb_ih, bb_hh,
                    W_out, b_out, transitions):
    sentence = np.asarray(sentence, dtype=np.int64)
    sentence_len = np.asarray(sentence_len, dtype=np.int64)
    nb, nt = sentence.shape
    emb = np.asarray(embed_table, np.float32)[sentence]          # [B,T,E]
    xf = (np.einsum("bte,ge->btg", emb, np.asarray(Wf_ih, np.float32))
          + bf_ih + bf_hh).astype(np.float32).transpose(1, 0, 2)
    xb = (np.einsum("bte,ge->btg", emb, np.asarray(Wb_ih, np.float32))
          + bb_ih + bb_hh).astype(np.float32).transpose(1, 0, 2)
    hf = _np_lstm(xf, np.asarray(Wf_hh, np.float32))
    hb = _np_lstm(xb[::-1], np.asarray(Wb_hh, np.float32))[::-1]
    hcat = np.concatenate([hf, hb], axis=-1)
    feats = (np.einsum("tbh,kh->tbk", hcat, np.asarray(W_out, np.float32))
             + b_out).astype(np.float32)                          # [T,B,K]
    trans = np.asarray(transitions, np.float32)
    dp = np.full((nb, K), NEG, np.float32)
    dp[:, START] = 0.0
    ident = np.arange(K)[None, :]
    bptrs = np.zeros((nt, nb, K), np.int32)
    for t in range(nt):
        mt = t < sentence_len
        scores = (dp[:, None, :] + trans[None, :, :]).astype(np.float32)
        bp = np.argmax(scores, axis=2)
        dpn = (np.max(scores, axis=2) + feats[t]).astype(np.float32)
        dp = np.where(mt[:, None], dpn, dp).astype(np.float32)
        bptrs[t] = np.where(mt[:, None], bp, ident)
    terminal = (dp + trans[STOP][None, :]).astype(np.float32)
    best = np.argmax(terminal, axis=1)
    scores_out = np.max(terminal, axis=1)
    path = np.zeros((nt, nb), np.int32)
    tag = best
    for t in range(nt - 1, -1, -1):
        path[t] = tag
        tag = bptrs[t][np.arange(nb), tag]
    return scores_out, np.ascontiguousarray(path.T).astype(np.int32)


def core_expected(cfg: Cfg, in_full, core):
    """Expected dp_all/ftb for one core (numpy), for sim validation."""
    dirn, bs = cfg.cores[core]
    T = cfg.T
    sentence = np.asarray(in_full["sentence"], np.int64)[bs:bs + B, :T]
    emb = np.asarray(in_full["embed_table"], np.float32)
    trans = np.asarray(in_full["transitions"], np.float32)
    ef = emb[sentence]                                          # [B,T,E]
    xf = (np.einsum("bte,ge->btg", ef, np.asarray(in_full["Wf_ih"], np.float32))
          + in_full["bf_ih"] + in_full["bf_hh"]).astype(np.float32).transpose(1, 0, 2)
    xb = (np.einsum("bte,ge->btg", ef, np.asarray(in_full["Wb_ih"], np.float32))
          + in_full["bb_ih"] + in_full["bb_hh"]).astype(np.float32).transpose(1, 0, 2)
    hf = _np_lstm(xf, np.asarray(in_full["Wf_hh"], np.float32))
    hb = _np_lstm(xb[::-1], np.asarray(in_full["Wb_hh"], np.float32))[::-1]
    W_out = np.asarray(in_full["W_out"], np.float32)
    pf = np.einsum("tbh,kh->tbk", hf, W_out[:, :H]).astype(np.float32)
    pb = np.einsum("tbh,kh->tbk", hb, W_out[:, H:]).astype(np.float32)
    feats = ((pf + pb) + np.asarray(in_full["b_out"], np.float32)).astype(np.float32)
    dp = np.full((B, K), NEG, np.float32)
    dp[:, START] = 0.0
    dp_all = np.zeros((B, T, K), np.float32)
    for t in range(T):
        scores = (dp[:, None, :] + trans[None, :, :]).astype(np.float32)
        dp = (np.max(scores, axis=2) + feats[t]).astype(np.float32)
        dp_all[:, t] = dp
    ftb = np.ascontiguousarray(feats.transpose(1, 0, 2))  # [B, T, K]
    return {
        "dp_all": dp_all.reshape(B, T * K),
        "ftb": ftb.reshape(B, T * K).astype(np.float32),
    }


# --------------------------------------------------------------------------
# top-level entry
# --------------------------------------------------------------------------

_CACHED = {}


def _build_full(cfg: Cfg):
    import concourse.mybir as mybir
    import concourse.tile as tile
    from concourse import bacc

    f32 = mybir.dt.float32
    i32 = mybir.dt.int32
    nc = bacc.Bacc(None, target_bir_lowering=False, debug=False,
                   num_devices=cfg.n_cores)
    ins = {}
    ins["table"] = nc.dram_tensor("table", [cfg.vocab, E], f32,
                                  kind="ExternalInput")[:]
    ins["idx"] = nc.dram_tensor("idx", [128, cfg.n_groups], i32,
                                kind="ExternalInput")[:]
    ins["w_ihT"] = nc.dram_tensor("w_ihT", [128, 2 * G], f32,
                                  kind="ExternalInput")[:]
    ins["w_hhT"] = nc.dram_tensor("w_hhT", [128, 2 * G], f32,
                                  kind="ExternalInput")[:]
    ins["bias_t"] = nc.dram_tensor("bias_t", [128, 8], f32,
                                   kind="ExternalInput")[:]
    ins["w_outT"] = nc.dram_tensor("w_outT", [128, 2 * K], f32,
                                   kind="ExternalInput")[:]
    ins["bout"] = nc.dram_tensor("bout", [B, K], f32, kind="ExternalInput")[:]
    ins["transB"] = nc.dram_tensor("transB", [B, K * K], f32,
                                   kind="ExternalInput")[:]
    ins["dp0"] = nc.dram_tensor("dp0", [B, K], f32, kind="ExternalInput")[:]
    outs = {
        "dp_all": nc.dram_tensor("dp_all", [B, cfg.T * K], f32,
                                 kind="ExternalOutput")[:],
        "ftb": nc.dram_tensor("ftb", [B, cfg.T * K], f32,
                              kind="ExternalOutput")[:],
    }
    import concourse.tile as tile_mod
    with tile_mod.TileContext(nc) as tc:
        build_body(nc, tc, ins, outs, cfg)
    nc.compile()
    return nc


def kernel(**inputs):
    sentence = np.asarray(inputs["sentence"])
    sentence_len = np.asarray(inputs["sentence_len"])
    if sentence.shape != (B_GLOB, T_FULL) or not np.all(sentence_len == T_FULL):
        return numpy_reference(**inputs)

    from concourse.bass_utils import run_bass_kernel_spmd

    cfg = Cfg()
    if "nc" not in _CACHED:
        _CACHED["nc"] = _build_full(cfg)
    nc = _CACHED["nc"]
    in_maps = prep_core_inputs(
        cfg, sentence, inputs["embed_table"],
        inputs["Wf_ih"], inputs["Wf_hh"], inputs["bf_ih"], inputs["bf_hh"],
        inputs["Wb_ih"], inputs["Wb_hh"], inputs["bb_ih"], inputs["bb_hh"],
        inputs["W_out"], inputs["b_out"], inputs["transitions"],
    )
    r = run_bass_kernel_spmd(nc, in_maps, core_ids=list(range(cfg.n_cores)))
    return postprocess(cfg, r.results, inputs["transitions"])
